# revision 2
# baseline (speedup 1.0000x reference)
"""Trainium2 Bass kernel for a 3-layer GCN (GCNConv x3 + global mean pool
+ linear head), 8-core SPMD.

Structure per layer: node-major transform (u = h @ W via lhsT=hT-block
matmuls, symmetric-norm coefficients folded into the segment-sum selection
matrices), chunked DMA of the u shard, 8-core AllGather of the u table,
then SWDGE dma_gather streams (4 queues, 4-deep buffering) feeding
PE segment-sum matmuls with per-supertile streamed coefficient tiles.
"""
import numpy as np
import sys

if "/opt/trn_rl_repo" not in sys.path:
    sys.path.insert(0, "/opt/trn_rl_repo")

import concourse.bass as bass
import concourse.bacc as bacc
import concourse.mybir as mybir
import concourse.tile as tile
from concourse.masks import make_identity
from concourse.bass_utils import run_bass_kernel_spmd

N, E, DIN, H, NGRAPH, OUT = 50000, 800000, 128, 128, 64, 8
NCORES = 8
SHARD = N // NCORES
M_COLS = 14
CHUNK_SLOTS = 128
ST_BINS = 36
PSUM_COLS = 512
GB_BINS = 8
NQ = 4
UC_ST = ST_BINS * M_COLS * 2 + 8     # per-supertile ucols cols (A|B|filler8)

F32 = mybir.dt.float32
I16 = mybir.dt.int16


# ----------------------------------------------------------------- host prep
def _preprocess(edge_index, batch):
    src = np.asarray(edge_index[0], dtype=np.int64)
    dst = np.asarray(edge_index[1], dtype=np.int64)
    batch = np.asarray(batch, dtype=np.int64)

    dst_counts = np.bincount(dst, minlength=N)
    deg = dst_counts.astype(np.float64) + 1.0
    dinv = 1.0 / np.sqrt(deg)
    cnt = np.bincount(batch, minlength=NGRAPH).astype(np.float64)
    inv_cnt = (1.0 / np.maximum(cnt, 1.0)).astype(np.float32)

    order = np.argsort(dst, kind="stable")
    src_sorted = src[order]
    dst_starts = np.zeros(N + 1, dtype=np.int64)
    np.cumsum(dst_counts, out=dst_starts[1:])

    lo_cnt = np.zeros(N, np.int64)
    hi_cnt = np.zeros(N, np.int64)
    src_is_lo = src < (N // 2)
    np.add.at(lo_cnt, dst[src_is_lo], 1)
    np.add.at(hi_cnt, dst[~src_is_lo], 1)
    self_lo = np.arange(N) < (N // 2)
    lo_cnt += self_lo
    hi_cnt += ~self_lo

    per_core_bins = []
    for c in range(NCORES):
        lo = c * SHARD
        nodes = np.arange(lo, lo + SHARD)
        d = (dst_counts[lo:lo + SHARD] + 1).astype(np.int64)
        order_d = np.argsort(-d, kind="stable")
        B = -(-SHARD // M_COLS)
        while True:
            bins_nodes = [[] for _ in range(B)]
            for r in range(M_COLS):
                idxs = order_d[r * B:(r + 1) * B]
                for i, oi in enumerate(idxs):
                    bi = (B - 1 - i) if (r % 2 == 1) else i
                    bins_nodes[bi].append(nodes[oi])
            load_lo = np.array([sum(lo_cnt[v] for v in bn) for bn in bins_nodes])
            load_hi = np.array([sum(hi_cnt[v] for v in bn) for bn in bins_nodes])
            for _ in range(40000):
                worst = np.maximum(load_lo, load_hi)
                hi_b = int(np.argmax(worst))
                if worst[hi_b] <= CHUNK_SLOTS:
                    break
                use_lo = load_lo[hi_b] >= load_hi[hi_b]
                cc = lo_cnt if use_lo else hi_cnt
                lo_b = int(np.argmin(np.maximum(load_lo, load_hi)))
                if lo_b == hi_b:
                    break
                vh = max(bins_nodes[hi_b], key=lambda v: cc[v])
                vl = min(bins_nodes[lo_b], key=lambda v: cc[v])
                if cc[vh] - cc[vl] <= 0:
                    break
                bins_nodes[hi_b].remove(vh); bins_nodes[hi_b].append(vl)
                bins_nodes[lo_b].remove(vl); bins_nodes[lo_b].append(vh)
                for arr, cnts in ((load_lo, lo_cnt), (load_hi, hi_cnt)):
                    arr[hi_b] += cnts[vl] - cnts[vh]
                    arr[lo_b] += cnts[vh] - cnts[vl]
            if max(np.maximum(load_lo, load_hi).max(), 0) <= CHUNK_SLOTS:
                break
            B = B + max(1, B // 100)
        per_core_bins.append(bins_nodes)

    nbins_max = max(len(b) for b in per_core_bins)
    NBINS = -(-nbins_max // ST_BINS) * ST_BINS
    NST = NBINS // ST_BINS
    P_pos = NST * PSUM_COLS
    TP = P_pos // 128
    HALF_ROW = (NCORES // 2) * P_pos
    NGI = -(-NBINS // GB_BINS)

    pos_of_node = np.full(N, -1, dtype=np.int64)
    core_of_node = np.full(N, -1, dtype=np.int64)
    for c in range(NCORES):
        for j, bn in enumerate(per_core_bins[c]):
            base = (j // ST_BINS) * PSUM_COLS + (j % ST_BINS) * M_COLS
            for t, v in enumerate(bn):
                pos_of_node[v] = base + t
                core_of_node[v] = c
    assert (pos_of_node >= 0).all()
    grow_of_node = core_of_node * P_pos + pos_of_node

    per_core = []
    for c in range(NCORES):
        bins_nodes = per_core_bins[c]
        flatA = np.zeros(NGI * GB_BINS * 128, np.int64)
        flatB = np.zeros(NGI * GB_BINS * 128, np.int64)
        ucols = np.zeros((CHUNK_SLOTS, NST * UC_ST), dtype=np.float32)
        for j, bn in enumerate(bins_nodes):
            st, k = divmod(j, ST_BINS)
            colA = st * UC_ST + k * M_COLS
            colB = st * UC_ST + ST_BINS * M_COLS + k * M_COLS
            sA = sB = 0
            for t, v in enumerate(bn):
                st0, en0 = dst_starts[v], dst_starts[v + 1]
                srcs = np.concatenate([src_sorted[st0:en0], [v]])
                coefs = (dinv[srcs] * dinv[v]).astype(np.float32)
                g = grow_of_node[srcs]
                is_lo = g < HALF_ROW
                glo, clo = g[is_lo], coefs[is_lo]
                ghi, chi = g[~is_lo] - HALF_ROW, coefs[~is_lo]
                flatA[j * 128 + sA: j * 128 + sA + len(glo)] = glo
                ucols[sA:sA + len(glo), colA + t] = clo
                sA += len(glo)
                flatB[j * 128 + sB: j * 128 + sB + len(ghi)] = ghi
                ucols[sB:sB + len(chi), colB + t] = chi
                sB += len(ghi)
            assert sA <= 128 and sB <= 128

        pmat = np.zeros((128, TP * NGRAPH), dtype=np.float32)
        node_order = np.zeros(P_pos, np.int64)
        has_node = np.zeros(P_pos, bool)
        mask = core_of_node == c
        vnodes = np.nonzero(mask)[0]
        vpos = pos_of_node[vnodes]
        pp, tt = vpos % 128, vpos // 128
        pmat[pp, tt * NGRAPH + batch[vnodes]] = inv_cnt[batch[vnodes]]
        node_order[vpos] = vnodes
        has_node[vpos] = True
        per_core.append(dict(flatA=flatA, flatB=flatB, ucols=ucols,
                             pmat=pmat, node_order=node_order,
                             has_node=has_node))

    meta = dict(NBINS=NBINS, NST=NST, P_pos=P_pos, TP=TP, NGI=NGI,
                HALF_ROW=HALF_ROW)
    return meta, per_core


# -------------------------------------------------------------- device build
def _build(meta, repeat=1, shared_ufull=False, g_bufs=4, s_bufs=3,
           gb_bins=GB_BINS, scratch=16384):
    NBINS, NST, P_pos, TP = meta["NBINS"], meta["NST"], meta["P_pos"], meta["TP"]
    NGI, HALF_ROW = meta["NGI"], meta["HALF_ROW"]

    nc = bacc.Bacc("TRN2", target_bir_lowering=False, debug=False,
                   num_devices=NCORES, num_swdge_queues=NQ,
                   dynamic_dma_scratch_size=scratch)

    xg_d = nc.dram_tensor("xg", [P_pos, 128], F32, kind="ExternalInput")
    wt_d = nc.dram_tensor("wt", [128, 3 * H], F32, kind="ExternalInput")
    wh_d = nc.dram_tensor("wh", [128, OUT], F32, kind="ExternalInput")
    bvec_d = nc.dram_tensor("bvec", [128, 3], F32, kind="ExternalInput")
    bhb_d = nc.dram_tensor("bhb", [NGRAPH, OUT], F32, kind="ExternalInput")
    NGI2 = -(-NBINS // gb_bins)
    GI_COLS = NGI2 * gb_bins * 8
    giA_d = nc.dram_tensor("gidxA", [128, GI_COLS], I16, kind="ExternalInput")
    giB_d = nc.dram_tensor("gidxB", [128, GI_COLS], I16, kind="ExternalInput")
    ucols_d = nc.dram_tensor("ucols", [128, NST * UC_ST], F32,
                             kind="ExternalInput")
    pmat_d = nc.dram_tensor("pmat", [128, TP * NGRAPH], F32, kind="ExternalInput")
    out_d = nc.dram_tensor("out", [NGRAPH, OUT], F32, kind="ExternalOutput")

    u_shard = nc.dram_tensor("u_shard", [P_pos, 128], F32)
    u_full = nc.dram_tensor(
        "u_full", [NCORES * P_pos, 128], F32,
        addr_space=("Shared" if shared_ufull else "Local"))
    ar_in = nc.dram_tensor("ar_in", [NGRAPH, OUT], F32)
    ar_out = nc.dram_tensor("ar_out", [NGRAPH, OUT], F32, addr_space="Shared")

    rg = [list(range(NCORES))]

    with tile.TileContext(nc) as tc:
        with (
            tc.tile_pool(name="const", bufs=1) as cpool,
            tc.tile_pool(name="unm", bufs=1) as upool,
            tc.tile_pool(name="uc", bufs=3) as ucpool,
            tc.tile_pool(name="GA", bufs=g_bufs) as gpoolA,
            tc.tile_pool(name="GB", bufs=g_bufs) as gpoolB,
            tc.tile_pool(name="small", bufs=2) as spool,
            tc.tile_pool(name="ps_tr", bufs=2, space="PSUM") as ps_tr,
            tc.tile_pool(name="ps_mm", bufs=2, space="PSUM") as ps_mm,
            tc.tile_pool(name="ps_s", bufs=s_bufs, space="PSUM") as ps_s,
            tc.tile_pool(name="ps_end", bufs=1, space="PSUM") as ps_end,
        ):
            # ---- constants
            wt = cpool.tile([128, 3 * H], F32)
            nc.sync.dma_start(wt[:], wt_d[:])
            wh = cpool.tile([128, OUT], F32)
            nc.sync.dma_start(wh[:], wh_d[:])
            bvec = cpool.tile([128, 3], F32)
            nc.sync.dma_start(bvec[:], bvec_d[:])
            bhb = cpool.tile([NGRAPH, OUT], F32)
            nc.sync.dma_start(bhb[:], bhb_d[:])
            giA = cpool.tile([128, GI_COLS], I16)
            nc.sync.dma_start(giA[:], giA_d[:])
            giB = cpool.tile([128, GI_COLS], I16)
            nc.sync.dma_start(giB[:], giB_d[:])
            pmat = cpool.tile([128, TP * NGRAPH], F32)
            nc.sync.dma_start(pmat[:], pmat_d[:])
            ident = cpool.tile([128, 128], F32)
            make_identity(nc, ident[:])
            hT = cpool.tile([128, P_pos], F32)

            # ---- load x node-major, transpose once into hT (feature-major)
            xg = upool.tile([128, TP * 128], F32, tag="unm")
            nc.sync.dma_start(
                xg[:].rearrange("p (t f) -> p t f", f=128),
                xg_d.ap().rearrange("(t p) f -> p t f", p=128))
            for t in range(TP):
                trp = ps_tr.tile([128, 128], F32, tag="tr")
                nc.tensor.transpose(trp[:], xg[:, t * 128:(t + 1) * 128], ident[:])
                nc.vector.tensor_copy(hT[:, t * 128:(t + 1) * 128], trp[:])

            # ---- layers
            qctr = 0
            for l in [ll for _ in range(repeat) for ll in range(3)]:
                # transform: u_nm[node, feat] = h @ W node-major; chunked DMA
                u_nm = upool.tile([128, TP * 128], F32, tag="unm")
                for g in range(NST):
                    psu = ps_mm.tile([128, PSUM_COLS], F32, tag="mm")
                    for i in range(PSUM_COLS // 128):
                        t = g * (PSUM_COLS // 128) + i
                        nc.tensor.matmul(
                            psu[:, i * 128:(i + 1) * 128],
                            lhsT=hT[:, t * 128:(t + 1) * 128],
                            rhs=wt[:, l * H:(l + 1) * H],
                            start=True, stop=True)
                    nc.vector.tensor_copy(
                        u_nm[:, g * PSUM_COLS:(g + 1) * PSUM_COLS], psu[:])
                    nc.sync.dma_start(
                        out=u_shard.ap()[g * PSUM_COLS:(g + 1) * PSUM_COLS, :]
                            .rearrange("(t p) f -> p t f", p=128),
                        in_=u_nm[:, g * PSUM_COLS:(g + 1) * PSUM_COLS]
                            .rearrange("p (t f) -> p t f", f=128))
                nc.gpsimd.collective_compute(
                    "AllGather", mybir.AluOpType.bypass, replica_groups=rg,
                    ins=[u_shard.ap().opt()], outs=[u_full.ap().opt()])

                uc_tiles = {}

                def fetch_uc(st):
                    tl = ucpool.tile([128, UC_ST], F32, tag="uc")
                    nc.sync.dma_start(
                        tl[:], ucols_d[:, st * UC_ST:(st + 1) * UC_ST])
                    uc_tiles[st] = tl

                fetch_uc(0)
                sps = None
                uc = None
                GA = GB = None
                for j in range(NBINS):
                    if j % gb_bins == 0:
                        b = j // gb_bins
                        n = min(gb_bins, NBINS - j)
                        coff = b * gb_bins * 8
                        GA = gpoolA.tile([128, gb_bins * 128], F32, tag="GA")
                        nc.gpsimd.dma_gather(
                            GA[:, :n * 128].rearrange("p (c f) -> p c f", f=128),
                            u_full[:HALF_ROW, :],
                            giA[:, coff:coff + n * 8],
                            n * 128, n * 128, 128, queue_num=qctr % NQ)
                        qctr += 1
                        GB = gpoolB.tile([128, gb_bins * 128], F32, tag="GB")
                        nc.gpsimd.dma_gather(
                            GB[:, :n * 128].rearrange("p (c f) -> p c f", f=128),
                            u_full[HALF_ROW:, :],
                            giB[:, coff:coff + n * 8],
                            n * 128, n * 128, 128, queue_num=qctr % NQ)
                        qctr += 1
                    st, k = divmod(j, ST_BINS)
                    if k == 0:
                        if st + 1 < NST:
                            fetch_uc(st + 1)
                        uc = uc_tiles.pop(st)
                        sps = ps_s.tile([128, PSUM_COLS], F32, tag="s")
                    jl = j % gb_bins
                    colA = k * M_COLS
                    colB = ST_BINS * M_COLS + k * M_COLS
                    nc.tensor.matmul(
                        sps[:, colA:colA + M_COLS],
                        lhsT=GA[:, jl * 128:(jl + 1) * 128],
                        rhs=uc[:, colA:colA + M_COLS],
                        start=True, stop=False)
                    nc.tensor.matmul(
                        sps[:, colA:colA + M_COLS],
                        lhsT=GB[:, jl * 128:(jl + 1) * 128],
                        rhs=uc[:, colB:colB + M_COLS],
                        start=False, stop=True)
                    if k == ST_BINS - 1:
                        nc.tensor.matmul(
                            sps[:, ST_BINS * M_COLS:PSUM_COLS],
                            lhsT=GB[:, jl * 128:(jl + 1) * 128],
                            rhs=uc[:, 2 * ST_BINS * M_COLS:UC_ST],
                            start=True, stop=True)
                        dst_sl = hT[:, st * PSUM_COLS:(st + 1) * PSUM_COLS]
                        if l < 2:
                            nc.scalar.activation(
                                dst_sl, sps[:],
                                mybir.ActivationFunctionType.Relu,
                                bias=bvec[:, l:l + 1])
                        else:
                            nc.vector.tensor_scalar_add(
                                dst_sl, sps[:], bvec[:, l:l + 1])

            # ---- global mean pool + head
            plp = ps_end.tile([NGRAPH, 128], F32, tag="pool")
            for t in range(TP):
                trp = ps_tr.tile([128, 128], F32, tag="tr")
                nc.tensor.transpose(trp[:], hT[:, t * 128:(t + 1) * 128], ident[:])
                h_nm = spool.tile([128, 128], F32, tag="hnm")
                nc.vector.tensor_copy(h_nm[:], trp[:])
                nc.tensor.matmul(
                    plp[:], lhsT=pmat[:, t * NGRAPH:(t + 1) * NGRAPH],
                    rhs=h_nm[:], start=(t == 0), stop=(t == TP - 1))
            pool_nm = spool.tile([NGRAPH, 128], F32, tag="plnm")
            nc.vector.tensor_copy(pool_nm[:], plp[:])
            trp2 = ps_tr.tile([128, NGRAPH], F32, tag="tr")
            nc.tensor.transpose(trp2[:], pool_nm[:], ident[:NGRAPH, :NGRAPH])
            poolT = spool.tile([128, NGRAPH], F32, tag="plT")
            nc.vector.tensor_copy(poolT[:], trp2[:])
            hdp = ps_end.tile([NGRAPH, OUT], F32, tag="head")
            nc.tensor.matmul(hdp[:], lhsT=poolT[:], rhs=wh[:], start=True,
                             stop=True)
            hd = spool.tile([NGRAPH, OUT], F32, tag="hd")
            nc.vector.tensor_copy(hd[:], hdp[:])
            nc.sync.dma_start(ar_in[:], hd[:])
            nc.gpsimd.collective_compute(
                "AllReduce", mybir.AluOpType.add, replica_groups=rg,
                ins=[ar_in.ap().opt()], outs=[ar_out.ap().opt()])
            res = spool.tile([NGRAPH, OUT], F32, tag="res")
            nc.sync.dma_start(res[:], ar_out[:])
            nc.vector.tensor_add(res[:], res[:], bhb[:])
            nc.sync.dma_start(out_d[:], res[:])

    nc.compile()
    return nc


_CACHE = {}


def _get_compiled(meta_key, meta):
    if meta_key not in _CACHE:
        _CACHE[meta_key] = _build(meta)
    return _CACHE[meta_key]


def wrap_stream(flat, nbins, gb):
    ngi = -(-nbins // gb)
    out = np.zeros((128, ngi * gb * 8), np.int16)
    for b in range(ngi):
        n = min(gb, nbins - b * gb)
        v = flat[b * gb * 128: b * gb * 128 + n * 128]
        w = v.reshape(n * 8, 16).T.astype(np.int16)
        out[:, b * gb * 8: b * gb * 8 + n * 8] = np.tile(w, (8, 1))
    return out


def make_in_maps(inputs, meta, per_core, gb_bins=GB_BINS):
    wt = np.concatenate([np.asarray(inputs["W0"], np.float32),
                         np.asarray(inputs["W1"], np.float32),
                         np.asarray(inputs["W2"], np.float32)], axis=1)
    bvec = np.stack([np.asarray(inputs["b0"], np.float32),
                     np.asarray(inputs["b1"], np.float32),
                     np.asarray(inputs["b2"], np.float32)], axis=1)
    wh = np.asarray(inputs["Wh"], np.float32)
    bhb = np.tile(np.asarray(inputs["bh"], np.float32)[None, :], (NGRAPH, 1))
    x = np.ascontiguousarray(np.asarray(inputs["x"], np.float32))

    in_maps = []
    for c in range(NCORES):
        pc = per_core[c]
        xg = np.zeros((meta["P_pos"], 128), np.float32)
        xg[pc["has_node"]] = x[pc["node_order"][pc["has_node"]]]
        in_maps.append({
            "xg": xg, "wt": wt, "wh": wh, "bvec": bvec, "bhb": bhb,
            "gidxA": wrap_stream(pc["flatA"], meta["NBINS"], gb_bins),
            "gidxB": wrap_stream(pc["flatB"], meta["NBINS"], gb_bins),
            "ucols": pc["ucols"], "pmat": pc["pmat"],
        })
    return in_maps


def kernel(x, edge_index, batch, W0, b0, W1, b1, W2, b2, Wh, bh, **_ignored):
    inputs = dict(x=x, W0=W0, b0=b0, W1=W1, b1=b1, W2=W2, b2=b2, Wh=Wh, bh=bh)
    meta, per_core = _preprocess(edge_index, batch)
    nc = _get_compiled((meta["NBINS"], meta["P_pos"]), meta)
    in_maps = make_in_maps(inputs, meta, per_core)
    res = run_bass_kernel_spmd(nc, in_maps, core_ids=list(range(NCORES)))
    return np.asarray(res.results[0]["out"], np.float32)
